# revision 1
# baseline (speedup 1.0000x reference)
"""Trainium2 Bass kernel for BigraphGATv2 (4-layer GATv2: 2 item-item + 2 user-item).

Design (8 NeuronCores, SPMD):
  - Nodes sharded by dst: core c owns nodes with n % 8 == c. Permuted global
    row id: (n % 8) * S_pad + n // 8. Edges live on the core owning their dst.
  - Per layer: dense phase computes XL~/XR~ tables for the core's shard
    ([S_pad, 132] rows: [XL~(128) | XL.att | 0 | 0.5-ish]), XL~ is AllGathered
    (gathers need arbitrary src rows), XR~ stays local (dst rows are local).
  - Edge phase: slots (edges incl. self-loops) sorted by dst, tiled into
    128-dst-node tiles; per tile: gather-chunks of 128 slots (z built by
    indirect gather-add of XL~[src] and XR~[dst] over an eattr*We prefill)
    plus one self-chunk (contiguous XL/XR tile loads, no gather).
  - Scores: leakyrelu(z)@att = 0.2*(z@att) + 0.8*(relu-pos - relu-neg) using
    |att|-prescaled, sign-sorted feature space (folded into weights on host);
    z@att decomposes linearly into table column 128. Segment softmax skips the
    max subtraction (scores bounded; exactly equivalent math).
  - Aggregation: one-hot Mexp matmul into PSUM accumulates sum(exp*z), segdot
    (col 129) and segsum (col 130); out = psum/segsum - xr - We~*segdot/segsum
    + bias. Output tiles are PE-transposed into the next layer's hT buffer.
"""
import numpy as np

P = 128
NC = 8
D = 128
W = 132          # table row width
N_ITEM = 100000
N_ALL = 150000
L = 4
NEG = 0.2

_cache = {}


def _plan_graph(edge_index, edge_attr, n_nodes):
    """Per-core slot tables for one graph. Returns dict with per-core tables
    and the shared chunk schedule."""
    s_real = n_nodes // NC
    s_pad = ((s_real + P - 1) // P) * P
    n_tiles = s_pad // P
    src = edge_index[0].astype(np.int64)
    dst = edge_index[1].astype(np.int64)
    ea = edge_attr[:, 0].astype(np.float32)

    cores = []
    for c in range(NC):
        m = (dst % NC) == c
        sc, dc, ec = src[m], dst[m], ea[m]
        srcg = (sc % NC) * s_pad + sc // NC     # global permuted row
        dstl = dc // NC                          # local row in this shard
        order = np.argsort(dstl, kind="stable")
        cores.append((srcg[order], dstl[order], ec[order]))

    # non-self slot counts per tile per core -> shared gather-chunk schedule
    gchunks = np.zeros(n_tiles, np.int64)
    for c in range(NC):
        _, dstl, _ = cores[c]
        cnt = np.bincount(dstl // P, minlength=n_tiles)
        gchunks = np.maximum(gchunks, (cnt + P - 1) // P)

    nch = int((gchunks + 1).sum())  # +1 self-chunk per tile
    # chunk schedule: for tile t: gchunks[t] gather chunks then 1 self chunk
    is_self = np.zeros(nch, bool)
    tile_of = np.zeros(nch, np.int64)
    j = 0
    for t in range(n_tiles):
        for _ in range(int(gchunks[t])):
            tile_of[j] = t; j += 1
        is_self[j] = True; tile_of[j] = t; j += 1
    assert j == nch

    tabs = []
    for c in range(NC):
        srcg, dstl, ec = cores[c]
        t_src = np.zeros((nch, P), np.int32)
        t_dst = np.zeros((nch, P), np.int32)
        t_ea = np.zeros((nch, P), np.float32)
        t_dl = np.full((nch, P), -1.0, np.float32)
        bounds = np.searchsorted(dstl, np.arange(0, s_pad + P, P))
        j = 0
        for t in range(n_tiles):
            lo, hi = bounds[t], bounds[t + 1]
            cnt = hi - lo
            g = int(gchunks[t])
            s, d, e = srcg[lo:hi], dstl[lo:hi], ec[lo:hi]
            for k in range(g):
                a, b = k * P, min((k + 1) * P, cnt)
                if b > a:
                    n = b - a
                    t_src[j, :n] = s[a:b]
                    t_dst[j, :n] = d[a:b]
                    t_ea[j, :n] = e[a:b]
                    t_dl[j, :n] = (d[a:b] - t * P).astype(np.float32)
                j += 1
            # self chunk
            t_dst[j, :] = t * P + np.arange(P)
            t_dl[j, :] = np.arange(P, dtype=np.float32)
            t_ea[j, :] = 1.0
            j += 1
        tabs.append(dict(src=t_src.T.copy(), dst=t_dst.T.copy(),
                         ea=t_ea.T.copy(), dl=t_dl.T.copy(),
                         dlr=t_dl.copy()))
    return dict(s_real=s_real, s_pad=s_pad, n_tiles=n_tiles, nch=nch,
                is_self=is_self, tile_of=tile_of, tabs=tabs)


def _fold_weights(Wl, bl, Wr, br, We, att, bias):
    """Per-layer host folding: feature permutation (att>=0 first) + |att| scale
    on the table space; input-side undo of previous layer's transform."""
    layers = []
    prev_perm, prev_s = None, None
    for l in range(L):
        a = att[l]
        perm = np.argsort(a < 0, kind="stable")
        c_pos = int((a >= 0).sum())
        s = np.abs(a[perm]).astype(np.float32)
        s = np.maximum(s, 1e-12)

        wl, wr = Wl[l].astype(np.float64), Wr[l].astype(np.float64)
        if prev_perm is not None:
            wl = wl[prev_perm, :] / prev_s[:, None]
            wr = wr[prev_perm, :] / prev_s[:, None]
        wla = wl @ a.astype(np.float64)
        wra = wr @ a.astype(np.float64)
        wlx = np.zeros((D, W), np.float32)
        wrx = np.zeros((D, W), np.float32)
        wlx[:, :D] = (wl[:, perm] * s[None, :]).astype(np.float32)
        wrx[:, :D] = (wr[:, perm] * s[None, :]).astype(np.float32)
        wlx[:, 128] = wla.astype(np.float32)
        wrx[:, 128] = wra.astype(np.float32)
        blx = np.zeros((1, W), np.float32)
        brx = np.zeros((1, W), np.float32)
        blx[0, :D] = bl[l][perm] * s
        brx[0, :D] = br[l][perm] * s
        blx[0, 128] = float(bl[l] @ a)
        brx[0, 128] = float(br[l] @ a)
        blx[0, 130] = 0.5
        brx[0, 130] = 0.5
        we = We[l][0]
        we_ext = np.zeros((P, W), np.float32)
        we_ext[:, :D] = (we[perm] * s)[None, :]
        we_ext[:, 128] = float(we @ a)
        we_ext[:, 129] = 1.0
        bias_full = np.zeros((P, W), np.float32)
        bias_full[:, :D] = (bias[l][perm] * s)[None, :]
        layers.append(dict(wlx=wlx, wrx=wrx, blx=blx, brx=brx, we=we_ext,
                           bias=bias_full, c_pos=c_pos, perm=perm, s=s))
        prev_perm, prev_s = perm, s
    return layers


def _build_program(plan_ii, plan_uiu):
    import sys
    sys.path.insert(0, "/opt/trn_rl_repo")
    import concourse.bass as bass
    import concourse.bacc as bacc
    import concourse.tile as tile
    from concourse import mybir

    F32, I32 = mybir.dt.float32, mybir.dt.int32
    AF = mybir.ActivationFunctionType
    ALU = mybir.AluOpType
    AP = bass.AP

    nc = bacc.Bacc("TRN2", target_bir_lowering=False, debug=False,
                   enable_asserts=True, num_devices=NC)

    sp1, sp2 = plan_ii["s_pad"], plan_uiu["s_pad"]
    plans = [plan_ii, plan_ii, plan_uiu, plan_uiu]

    # ---- IO ----
    ins = {}
    def inp(name, shape, dt=F32):
        ins[name] = nc.dram_tensor(name, shape, dt, kind="ExternalInput")
        return ins[name]

    xiT = inp("xiT", [P, sp1])
    xuT = inp("xuT", [P, sp2 - N_ITEM // NC])
    for l in range(L):
        inp(f"wlx{l}", [D, W]); inp(f"wrx{l}", [D, W])
        inp(f"blx{l}", [1, W]); inp(f"brx{l}", [1, W])
        inp(f"we{l}", [P, W]); inp(f"biasf{l}", [P, W])
        pl = plans[l]
        inp(f"src{l}", [P, pl["nch"]], I32)
        inp(f"dst{l}", [P, pl["nch"]], I32)
        inp(f"ea{l}", [P, pl["nch"]])
        inp(f"dl{l}", [P, pl["nch"]])
        inp(f"dlr{l}", [pl["nch"], P])
    inp("iota", [P, P])
    inp("iotac", [P, 1])
    inp("ident", [P, P])
    inp("nident", [P, P])

    out_nm = nc.dram_tensor("out_nm", [sp2, D], F32, kind="ExternalOutput")
    import os as _os
    PROBE = _os.environ.get("K_PROBE") == "1"
    if PROBE:
        p_xl = nc.dram_tensor("p_xl", [P, W], F32, kind="ExternalOutput")
        p_xlf = nc.dram_tensor("p_xlf", [P, W], F32, kind="ExternalOutput")
        p_z = nc.dram_tensor("p_z", [P, W], F32, kind="ExternalOutput")
        p_zs = nc.dram_tensor("p_zs", [P, W], F32, kind="ExternalOutput")
        p_e = nc.dram_tensor("p_e", [P, 512], F32, kind="ExternalOutput")
        p_ps = nc.dram_tensor("p_ps", [P, W], F32, kind="ExternalOutput")
        p_ht = nc.dram_tensor("p_ht", [P, P], F32, kind="ExternalOutput")

    # internal DRAM
    hT = [None] * (L + 1)
    hT[1] = nc.dram_tensor("hT1", [P, sp1], F32, kind="Internal")
    hT[2] = nc.dram_tensor("hT2", [P, sp2], F32, kind="Internal")
    hT[3] = nc.dram_tensor("hT3", [P, sp2], F32, kind="Internal")
    xlloc = [nc.dram_tensor(f"xlloc{l}", [plans[l]["s_pad"], W], F32, kind="Internal")
             for l in range(L)]
    xrloc = [nc.dram_tensor(f"xrloc{l}", [plans[l]["s_pad"], W], F32, kind="Internal")
             for l in range(L)]
    xlfull = [nc.dram_tensor(f"xlfull{l}", [NC * plans[l]["s_pad"], W], F32,
                             kind="Internal", addr_space="Shared")
              for l in range(L)]

    c_pos_list = _build_program.c_pos_list

    with tile.TileContext(nc) as tc:
        with tc.tile_pool(name="const", bufs=1) as cp, \
             tc.tile_pool(name="wts", bufs=1) as wp, \
             tc.tile_pool(name="tabs", bufs=1) as tp, \
             tc.tile_pool(name="dense", bufs=3) as dp, \
             tc.tile_pool(name="edge", bufs=12) as ep, \
             tc.tile_pool(name="etab", bufs=2) as etp, \
             tc.tile_pool(name="tile", bufs=3) as tlp, \
             tc.tile_pool(name="psA", bufs=2, space="PSUM") as psA, \
             tc.tile_pool(name="psB", bufs=2, space="PSUM") as psB, \
             tc.tile_pool(name="psD", bufs=1, space="PSUM") as psD:

            iotac_t = cp.tile([P, 1], F32, tag="iotac")
            nc.sync.dma_start(iotac_t[:], ins["iotac"][:, :])
            iota_t = cp.tile([P, P], F32, tag="iota")
            ident_t = cp.tile([P, P], F32, tag="ident")
            nident_t = cp.tile([P, P], F32, tag="nident")
            ones1_t = cp.tile([1, P], F32, tag="ones1")
            nc.vector.memset(ones1_t[:], 1.0)
            nc.sync.dma_start(iota_t[:], ins["iota"][:, :])
            nc.sync.dma_start(ident_t[:], ins["ident"][:, :])
            nc.sync.dma_start(nident_t[:], ins["nident"][:, :])

            # copy user cols of x~T into hT2
            nc.sync.dma_start(hT[2][:, N_ITEM // NC:], ins["xuT"][:, :])

            for l in range(L):
                pl = plans[l]
                sp = pl["s_pad"]; ntl = pl["n_tiles"]; nchl = pl["nch"]
                hin = ins["xiT"] if l == 0 else hT[l]
                first_uiu = (l == 2)
                last = (l == L - 1)

                # --- weights/consts for this layer ---
                wlx_t = wp.tile([D, W], F32, tag="wlx")
                wrx_t = wp.tile([D, W], F32, tag="wrx")
                blx_t = wp.tile([1, W], F32, tag="blx")
                brx_t = wp.tile([1, W], F32, tag="brx")
                we_t = wp.tile([P, W], F32, tag="we")
                biasf_t = wp.tile([P, W], F32, tag="biasf")
                nc.sync.dma_start(wlx_t[:], ins[f"wlx{l}"][:, :])
                nc.sync.dma_start(wrx_t[:], ins[f"wrx{l}"][:, :])
                nc.sync.dma_start(blx_t[:], ins[f"blx{l}"][:, :])
                nc.sync.dma_start(brx_t[:], ins[f"brx{l}"][:, :])
                nc.sync.dma_start(we_t[:], ins[f"we{l}"][:, :])
                nc.sync.dma_start(biasf_t[:], ins[f"biasf{l}"][:, :])

                # --- dense phase: XL~/XR~ for own shard ---
                for t in range(ntl):
                    ht_t = dp.tile([P, P], F32, tag="ht")
                    nc.sync.dma_start(ht_t[:], hin[:, t * P:(t + 1) * P])
                    pxl = psD.tile([P, W], F32, tag="pxl")
                    pxr = psD.tile([P, W], F32, tag="pxr")
                    nc.tensor.matmul(out=pxl[:], lhsT=ht_t[:], rhs=wlx_t[:],
                                     start=True, stop=False)
                    nc.tensor.matmul(out=pxl[:], lhsT=ones1_t[:], rhs=blx_t[:],
                                     start=False, stop=True)
                    nc.tensor.matmul(out=pxr[:], lhsT=ht_t[:], rhs=wrx_t[:],
                                     start=True, stop=False)
                    nc.tensor.matmul(out=pxr[:], lhsT=ones1_t[:], rhs=brx_t[:],
                                     start=False, stop=True)
                    xl_sb = dp.tile([P, W], F32, tag="xlsb")
                    xr_sb = dp.tile([P, W], F32, tag="xrsb")
                    nc.scalar.copy(out=xl_sb[:], in_=pxl[:])
                    nc.scalar.copy(out=xr_sb[:], in_=pxr[:])
                    nc.sync.dma_start(xlloc[l][t * P:(t + 1) * P, :], xl_sb[:])
                    nc.sync.dma_start(xrloc[l][t * P:(t + 1) * P, :], xr_sb[:])

                if PROBE and l == 0:
                    pxl_sb = dp.tile([P, W], F32, tag="probe1")
                    nc.sync.dma_start(pxl_sb[:], xlloc[l][0:P, :])
                    nc.sync.dma_start(p_xl[:, :], pxl_sb[:])

                # --- allgather XL~ ---
                nc.gpsimd.collective_compute(
                    "AllGather", ALU.bypass, replica_groups=[list(range(NC))],
                    ins=[xlloc[l][:, :]], outs=[xlfull[l][:, :]])

                # --- edge-phase tables resident in SBUF ---
                src_t = tp.tile([P, nchl], I32, tag=f"src{l % 2}")
                dst_t = tp.tile([P, nchl], I32, tag=f"dst{l % 2}")
                ea_t = tp.tile([P, nchl], F32, tag=f"ea{l % 2}")
                dl_t = tp.tile([P, nchl], F32, tag=f"dl{l % 2}")
                nc.sync.dma_start(src_t[:], ins[f"src{l}"][:, :])
                nc.sync.dma_start(dst_t[:], ins[f"dst{l}"][:, :])
                nc.sync.dma_start(ea_t[:], ins[f"ea{l}"][:, :])
                nc.sync.dma_start(dl_t[:], ins[f"dl{l}"][:, :])
                epos_t = tp.tile([P, nchl], F32, tag=f"epos{l % 2}")
                eneg_t = tp.tile([P, nchl], F32, tag=f"eneg{l % 2}")
                zlin_t = tp.tile([P, nchl], F32, tag=f"zlin{l % 2}")
                expe_t = tp.tile([P, nchl], F32, tag=f"expe{l % 2}")

                c_pos = c_pos_list[l]
                if PROBE and l == 0:
                    pxlf_sb = dp.tile([P, W], F32, tag="probe2")
                    nc.sync.dma_start(pxlf_sb[:], xlfull[l][7 * sp:7 * sp + P, :])
                    nc.sync.dma_start(p_xlf[:, :], pxlf_sb[:])

                # --- edge phase ---
                tile_chunks = [[] for _ in range(ntl)]
                for j in range(nchl):
                    tile_chunks[pl["tile_of"][j]].append(j)

                def score_chunk(j, z_t):
                    scratch = ep.tile([P, P], F32, tag="scr")
                    if c_pos > 0:
                        nc.scalar.activation(out=scratch[:, 0:c_pos],
                                             in_=z_t[:, 0:c_pos], func=AF.Relu,
                                             accum_out=epos_t[:, j:j + 1])
                    else:
                        nc.vector.memset(epos_t[:, j:j + 1], 0.0)
                    if c_pos < D:
                        nc.scalar.activation(out=scratch[:, 0:D - c_pos],
                                             in_=z_t[:, c_pos:D], func=AF.Relu,
                                             accum_out=eneg_t[:, j:j + 1])
                    else:
                        nc.vector.memset(eneg_t[:, j:j + 1], 0.0)
                    nc.vector.tensor_copy(out=zlin_t[:, j:j + 1], in_=z_t[:, 128:129])

                # stage 1: build z, scores for all chunks (z tiles kept in pool)
                z_tiles = {}
                exp_done = -1

                def flush_exp(hi):
                    nonlocal exp_done
                    lo = exp_done + 1
                    if hi < lo:
                        return
                    sl = slice(lo, hi + 1)
                    d1 = etp.tile([P, nchl], F32, tag="d1")
                    nc.vector.tensor_tensor(out=d1[:, sl], in0=epos_t[:, sl],
                                            in1=eneg_t[:, sl], op=ALU.subtract)
                    nc.vector.tensor_scalar(out=d1[:, sl], in0=d1[:, sl],
                                            scalar1=4.0, scalar2=None, op0=ALU.mult)
                    nc.vector.tensor_tensor(out=d1[:, sl], in0=d1[:, sl],
                                            in1=zlin_t[:, sl], op=ALU.add)
                    nc.scalar.activation(out=expe_t[:, sl], in_=d1[:, sl],
                                         func=AF.Exp, scale=NEG)
                    exp_done = hi

                for t in range(ntl):
                    chs = tile_chunks[t]
                    xrt = tlp.tile([P, W], F32, tag="xrt")
                    nc.sync.dma_start(xrt[:], xrloc[l][t * P:(t + 1) * P, :])
                    # build z for each chunk of this tile
                    for j in chs:
                        z_t = ep.tile([P, W], F32, tag="z")
                        if pl["is_self"][j]:
                            xlt = ep.tile([P, W], F32, tag="xlt")
                            nc.sync.dma_start(xlt[:], xlloc[l][t * P:(t + 1) * P, :])
                            nc.vector.tensor_tensor(out=z_t[:], in0=xlt[:],
                                                    in1=xrt[:], op=ALU.add)
                            nc.vector.tensor_tensor(out=z_t[:], in0=z_t[:],
                                                    in1=we_t[:], op=ALU.add)
                        else:
                            # one-hot expansion of xr rows: psum_exp[s,f] = xrt[dstloc[s], f]
                            dlr_b = ep.tile([P, P], F32, tag="dlrb")
                            nc.sync.dma_start(
                                dlr_b[:],
                                AP(ins[f"dlr{l}"][:, :].tensor, j * P,
                                   [[0, P], [1, P]]))
                            m01 = ep.tile([P, P], F32, tag="m01")
                            nc.vector.tensor_scalar(out=m01[:], in0=dlr_b[:],
                                                    scalar1=iotac_t[:, :],
                                                    scalar2=None, op0=ALU.is_equal)
                            pexp = psB.tile([P, W], F32, tag="exp")
                            nc.tensor.matmul(out=pexp[:], lhsT=m01[:],
                                             rhs=xrt[:], start=True, stop=True)
                            nc.vector.tensor_scalar(out=z_t[:], in0=we_t[:],
                                                    scalar1=ea_t[:, j:j + 1],
                                                    scalar2=None, op0=ALU.mult)
                            nc.gpsimd.indirect_dma_start(
                                out=z_t[:], out_offset=None,
                                in_=xlfull[l][:, :],
                                in_offset=bass.IndirectOffsetOnAxis(
                                    ap=src_t[:, j:j + 1], axis=0),
                                compute_op=ALU.add)
                            nc.vector.tensor_tensor(out=z_t[:], in0=z_t[:],
                                                    in1=pexp[:], op=ALU.add)
                        if PROBE and l == 0 and j == 0:
                            nc.sync.dma_start(p_z[:, :], z_t[:])
                        if PROBE and l == 0 and pl["is_self"][j] and pl["tile_of"][j] == 0:
                            nc.sync.dma_start(p_zs[:, :], z_t[:])
                        score_chunk(j, z_t)
                        z_tiles[j] = z_t
                    flush_exp(chs[-1])
                    # aggregate
                    pagg = psA.tile([P, W], F32, tag="agg")
                    for k, j in enumerate(chs):
                        mexp = ep.tile([P, P], F32, tag="mexp")
                        nc.vector.tensor_scalar(out=mexp[:], in0=iota_t[:],
                                                scalar1=dl_t[:, j:j + 1],
                                                scalar2=expe_t[:, j:j + 1],
                                                op0=ALU.is_equal, op1=ALU.mult)
                        nc.tensor.matmul(out=pagg[:], lhsT=mexp[:],
                                         rhs=z_tiles[j][:],
                                         start=(k == 0), stop=(k == len(chs) - 1))
                    for j in chs:
                        del z_tiles[j]
                    if PROBE and l == 0 and t == 0:
                        pps_sb = tlp.tile([P, W], F32, tag="probe3")
                        nc.scalar.copy(out=pps_sb[:], in_=pagg[:])
                        nc.sync.dma_start(p_ps[:, :], pps_sb[:])
                    # corrections
                    recip = tlp.tile([P, 1], F32, tag="recip")
                    sdr = tlp.tile([P, 1], F32, tag="sdr")
                    o1 = tlp.tile([P, P], F32, tag="o1")
                    wcor = tlp.tile([P, P], F32, tag="wcor")
                    nc.vector.reciprocal(out=recip[:], in_=pagg[:, 130:131])
                    nc.vector.tensor_tensor(out=sdr[:], in0=pagg[:, 129:130],
                                            in1=recip[:], op=ALU.mult)
                    nc.scalar.activation(out=o1[:], in_=pagg[:, 0:D],
                                         func=AF.Copy, scale=recip[:, :])
                    nc.vector.tensor_scalar(out=wcor[:], in0=we_t[:, 0:D],
                                            scalar1=sdr[:, :], scalar2=None,
                                            op0=ALU.mult)
                    if last:
                        o = tlp.tile([P, P], F32, tag="o")
                        nc.vector.tensor_tensor(out=o[:], in0=o1[:],
                                                in1=xrt[:, 0:D], op=ALU.subtract)
                        nc.vector.tensor_tensor(out=o[:], in0=o[:],
                                                in1=wcor[:], op=ALU.subtract)
                        nc.vector.tensor_tensor(out=o[:], in0=o[:],
                                                in1=biasf_t[:, 0:D], op=ALU.add)
                        nc.sync.dma_start(out_nm[t * P:(t + 1) * P, :], o[:])
                    else:
                        ptr = psB.tile([P, P], F32, tag="tr")
                        nc.tensor.matmul(out=ptr[:], lhsT=o1[:], rhs=ident_t[:],
                                         start=True, stop=False)
                        nc.tensor.matmul(out=ptr[:], lhsT=xrt[:, 0:D],
                                         rhs=nident_t[:], start=False, stop=False)
                        nc.tensor.matmul(out=ptr[:], lhsT=wcor[:],
                                         rhs=nident_t[:], start=False, stop=False)
                        nc.tensor.matmul(out=ptr[:], lhsT=biasf_t[:, 0:D],
                                         rhs=ident_t[:], start=False, stop=True)
                        oT = tlp.tile([P, P], F32, tag="oT")
                        nc.scalar.copy(out=oT[:], in_=ptr[:])
                        # destination columns in next hT buffer
                        if l == 1:
                            lo = t * P
                            hi = min((t + 1) * P, N_ITEM // NC)
                            if hi > lo:
                                nc.sync.dma_start(hT[2][:, lo:hi],
                                                  oT[:, 0:hi - lo])
                        else:
                            nc.sync.dma_start(hT[l + 1][:, t * P:(t + 1) * P], oT[:])
                        if PROBE and l == 0 and t == 0:
                            nc.sync.dma_start(p_ht[:, :], oT[:])
                if PROBE and l == 0:
                    npe = min(512, nchl)
                    nc.sync.dma_start(p_e[:, 0:npe], expe_t[:, 0:npe])

    nc.compile()
    return nc, ins


def kernel(**inputs):
    x = np.asarray(inputs["x"], np.float32)
    eii = np.asarray(inputs["edge_index_ii"])
    aii = np.asarray(inputs["edge_attr_ii"], np.float32)
    euiu = np.asarray(inputs["edge_index_uiu"])
    auiu = np.asarray(inputs["edge_attr_uiu"], np.float32)
    n_item = int(inputs["n_item"])
    assert n_item == N_ITEM and x.shape == (N_ALL, D)
    Wl = np.asarray(inputs["Wl"], np.float32); bl = np.asarray(inputs["bl"], np.float32)
    Wr = np.asarray(inputs["Wr"], np.float32); br = np.asarray(inputs["br"], np.float32)
    We = np.asarray(inputs["We"], np.float32); att = np.asarray(inputs["att"], np.float32)
    bias = np.asarray(inputs["bias"], np.float32)

    plan_ii = _plan_graph(eii, aii, N_ITEM)
    plan_uiu = _plan_graph(euiu, auiu, N_ALL)
    layers = _fold_weights(Wl, bl, Wr, br, We, att, bias)

    key = ("prog", plan_ii["nch"], plan_uiu["nch"],
           tuple(ly["c_pos"] for ly in layers),
           tuple(plan_ii["tile_of"].tolist()), tuple(plan_uiu["tile_of"].tolist()))
    key = hash(key)
    if key not in _cache:
        _build_program.c_pos_list = [ly["c_pos"] for ly in layers]
        _cache[key] = _build_program(plan_ii, plan_uiu)
    nc, _ = _cache[key]

    sp1, sp2 = plan_ii["s_pad"], plan_uiu["s_pad"]
    sr1 = plan_ii["s_real"]

    iota = np.tile(np.arange(P, dtype=np.float32)[None, :], (P, 1))
    ident = np.eye(P, dtype=np.float32)

    # per-core inputs
    perm1, s1 = layers[1]["perm"], layers[1]["s"]
    in_maps = []
    for c in range(NC):
        im = {}
        xi = x[:N_ITEM][np.arange(c, N_ITEM, NC)]           # [12500, D]
        xiT = np.zeros((P, sp1), np.float32)
        xiT[:, :xi.shape[0]] = xi.T
        im["xiT"] = xiT
        xu = x[N_ITEM:][np.arange(c, N_ALL - N_ITEM, NC)]   # [6250, D]
        xut = (xu[:, perm1] * s1[None, :])                  # T1 transform
        xuT = np.zeros((P, sp2 - N_ITEM // NC), np.float32)
        xuT[:, :xut.shape[0]] = xut.T
        im["xuT"] = xuT
        for l in range(L):
            ly = layers[l]
            pl = plan_ii if l < 2 else plan_uiu
            im[f"wlx{l}"] = ly["wlx"]; im[f"wrx{l}"] = ly["wrx"]
            im[f"blx{l}"] = ly["blx"]; im[f"brx{l}"] = ly["brx"]
            im[f"we{l}"] = ly["we"]; im[f"biasf{l}"] = ly["bias"]
            tb = pl["tabs"][c]
            im[f"src{l}"] = tb["src"]; im[f"dst{l}"] = tb["dst"]
            im[f"ea{l}"] = tb["ea"]; im[f"dl{l}"] = tb["dl"]
            im[f"dlr{l}"] = tb["dlr"]
        im["iota"] = iota; im["ident"] = ident; im["nident"] = -ident
        im["iotac"] = np.arange(P, dtype=np.float32)[:, None]
        in_maps.append(im)

    import sys
    sys.path.insert(0, "/opt/trn_rl_repo")
    from concourse.bass_utils import run_bass_kernel_spmd
    res = run_bass_kernel_spmd(nc, in_maps, core_ids=list(range(NC)))
    _cache["last_res"] = res

    # assemble + undo T3 transform
    perm3, s3 = layers[3]["perm"], layers[3]["s"]
    out = np.zeros((N_ALL, D), np.float32)
    for c in range(NC):
        o = res.results[c]["out_nm"]           # [sp2, D] in ~3 space
        rows = np.arange(c, N_ALL, NC)
        ot = o[:len(rows)] / s3[None, :]
        tmp = np.zeros((len(rows), D), np.float32)
        tmp[:, perm3] = ot
        out[rows] = tmp
    return out

